# revision 19
# baseline (speedup 1.0000x reference)
"""Trainium2 Bass kernel for the bipartite GNN recommender (8 NeuronCores).

v2 — redesigned from the profiled baseline (1.91ms):
- dest sub-tiles of 32 (dense region A) / 64 (region B) nodes stacked
  4/2-per-128-partition PSUM quad: cuts one-hot LDWEIGHTS + DVE build ~4x
- conv2 fused with pred_W1 (host folds conv2_W @ pred_W1 halves; |W2| and
  its sign permutation folded into the ts tables so P7's W2 multiply
  becomes two sign-split reduces)
- own-node y rows stay in SBUF (no self-loop indirect gathers)
- gather offsets loaded once, shared by both conv layers; msg gathers
  batched per 8 dest quads
- AllGathers shrunk: AG1/AG2 user region only, AG3 products only (fired
  right after P1, hidden under conv1/conv2); P7 edges resharded by u%8 so
  the user half of the pair MLP reads a locally-written table
- fp8 tables lambda-scaled out of the subnormal range
"""
import numpy as np
import ml_dtypes

from concourse import bass, mybir, tile
from concourse.bass import AP, IndirectOffsetOnAxis
from concourse.bass_utils import run_bass_kernel_spmd
from concourse.masks import make_identity
from concourse.tile import add_dep_helper

F32 = mybir.dt.float32
BF16 = mybir.dt.bfloat16
F8 = mybir.dt.float8e4
I32 = mybir.dt.int32
I8 = mybir.dt.int8

AF = mybir.ActivationFunctionType
ALU = mybir.AluOpType

N_CORES = 8
NU, NP, NE = 200000, 100000, 1000000
SHARD = 37760          # 25088 user rows + 12672 product rows per core
NQ_U = 196             # user quads (128 rows each) per core
NQ_A = 98              # region A quads (table split A)
NT = 295               # total quads incl products
UROWS = 25088
PROWS = 12672
TABU = 200704          # user table rows (8 * 25088)
L1, L2, L3 = 32.0, 131072.0, 131072.0
GQ = 8                 # dest quads per conv batch
G7 = 4                 # gather-chunk groups per P7 iteration

BF = ml_dtypes.bfloat16


# --------------------------------------------------------------------------
# legalization: this walrus build allows at most 1 sync wait per instruction
# --------------------------------------------------------------------------
def _split_sync_waits(nc, max_waits=1):
    import bass_rust
    for bb in nc.main_func.blocks:
        out = []
        for inst in bb.instructions:
            si = inst.sync_info
            if si is not None and si.on_wait is not None and len(si.on_wait) > max_waits:
                waits = list(si.on_wait)
                keep, extra = waits[-max_waits:], waits[:-max_waits]
                while extra:
                    chunk, extra = extra[:max_waits], extra[max_waits:]
                    nop = bass_rust.InstNoOp(name=f"I-{nc.next_id()}", ins=[], outs=[])
                    nop.engine = inst.engine
                    nop.bass_nofuse = True
                    nop.sync_info = mybir.SyncInfo(on_wait=chunk, on_update=[])
                    nc.register_instruction(nop, overwrite=True)
                    out.append(nop)
                si.on_wait = keep
            out.append(inst)
        del bb.instructions[:]
        for i in out:
            bb.add_instruction(i)


# --------------------------------------------------------------------------
# host-side sharding / layout prep
# --------------------------------------------------------------------------
def _pi_map(j):
    j = np.asarray(j, np.int64)
    c = j % 8
    l = j // 8
    return np.where(l < 12544, c * 12544 + l,
                    100352 + c * 12544 + (l - 12544)).astype(np.int32)


def _prepare(inputs):
    ei = np.asarray(inputs["edge_index"])
    u_idx = ei[0].astype(np.int64)
    p_idx = ei[1].astype(np.int64)

    # ---- conv edges: directed both ways, dest-sharded ----
    src = np.concatenate([u_idx, p_idx])
    dst = np.concatenate([p_idx, u_idx])
    core = (dst % 8).astype(np.int64)
    l = dst // 8
    quad = l >> 7
    sub = quad * 4 + ((l >> 5) & 3)
    NSUB = NQ_U * 4
    colw = (l & 31).astype(np.int8)
    srcpi = _pi_map(src)

    gkey = core * NSUB + sub
    order = np.argsort(gkey, kind="stable")
    gkey_s = gkey[order]
    pi_s = srcpi[order]
    colw_s = colw[order]

    cnt = np.bincount(gkey, minlength=8 * NSUB).reshape(8, NSUB)
    nch = np.maximum((cnt.max(axis=0) + 127) // 128, 1).astype(np.int64)
    chunk_start = np.zeros(NSUB + 1, np.int64)
    chunk_start[1:] = np.cumsum(nch)
    NCHUNKS = int(chunk_start[-1])

    starts = np.searchsorted(gkey_s, np.arange(8 * NSUB))
    pos = np.arange(len(gkey_s)) - starts[gkey_s]
    csub = gkey_s % NSUB
    ccore = gkey_s // NSUB
    ch = chunk_start[csub] + (pos >> 7)
    rows = np.zeros((8, 128, NCHUNKS), np.int32)
    cols = np.full((8, 128, NCHUNKS), -1, np.int8)
    rows[ccore, pos & 127, ch] = pi_s
    cols[ccore, pos & 127, ch] = colw_s

    # quad -> (subtile width, [(chunk_start, nchunks)]) — same for all cores
    quad_subs = []
    for q in range(NQ_U):
        subs = [q * 4 + i for i in range(4)]
        quad_subs.append((32, [(int(chunk_start[s]), int(nch[s])) for s in subs]))

    # ---- degrees / dis ----
    ncnt = np.bincount(dst, minlength=NU + NP).astype(np.float64)
    dis_all = (1.0 / np.sqrt(ncnt + 1.0)).astype(np.float32)

    # ---- P7: edges sharded by u%8 ----
    owner = (u_idx % 8).astype(np.int64)
    ordP = np.argsort(owner, kind="stable")
    cntP = np.bincount(owner, minlength=8)
    NCH7 = int((int(cntP.max()) + 127) // 128)
    NCH7 = ((NCH7 + (8 * G7) - 1) // (8 * G7)) * (8 * G7)
    offU_all = (u_idx // 8).astype(np.int32)
    offP_all = (UROWS + (p_idx % 8) * PROWS + p_idx // 8).astype(np.int32)
    ownerP_starts = np.zeros(9, np.int64)
    ownerP_starts[1:] = np.cumsum(cntP)

    # ---- weights host folds ----
    W_uf = np.asarray(inputs["W_uf"], np.float32)
    W_pf = np.asarray(inputs["W_pf"], np.float32)
    b_uf = np.asarray(inputs["b_uf"], np.float32)
    b_pf = np.asarray(inputs["b_pf"], np.float32)
    W1c = np.asarray(inputs["conv1_W"], np.float32)
    b1 = np.asarray(inputs["conv1_b"], np.float32)
    W2c = np.asarray(inputs["conv2_W"], np.float32)
    b2 = np.asarray(inputs["conv2_b"], np.float32)
    pW1 = np.asarray(inputs["pred_W1"], np.float32)
    pb1 = np.asarray(inputs["pred_b1"], np.float32)
    pW2 = np.asarray(inputs["pred_W2"], np.float32).reshape(64)
    pb2 = float(np.asarray(inputs["pred_b2"]).reshape(()))

    perm = np.argsort(pW2 < 0, kind="stable")
    PPOS = int((pW2 >= 0).sum())
    absw = np.abs(pW2)[perm]
    M1f = (L2 * (W2c @ pW1[:64])[:, perm] * absw[None, :]).astype(BF)
    M2f = (L3 * (W2c @ pW1[64:])[:, perm] * absw[None, :]).astype(BF)
    constu_f = (L3 * (((b2 @ pW1[:64]) + pb1)[perm] * absw)).astype(np.float32)
    constp_f = (L3 * ((b2 @ pW1[64:])[perm] * absw)).astype(np.float32)

    fw = np.ascontiguousarray(np.asarray(inputs["user_features"], np.float32))
    pw = np.ascontiguousarray(np.asarray(inputs["product_features"], np.float32))
    ue = np.asarray(inputs["user_emb"], np.float32)
    pe = np.asarray(inputs["product_emb"], np.float32)

    per_core = []
    for c in range(N_CORES):
        featT = np.zeros((128, SHARD), BF)
        embT = np.zeros((64, SHARD), BF)
        featT[:, :25000] = fw[c::8].T
        featT[:, 25088:37588] = pw[c::8].T
        embT[:, :25000] = ue[c::8].T
        embT[:, 25088:37588] = pe[c::8].T

        # dis per local user row: 4 blocks [L1*dis | dis/L1 | dis | (L3/L2)*dis]
        lids = np.arange(UROWS)
        nid = np.minimum(c + 8 * lids, NU + NP - 1)
        dvals = dis_all[nid].astype(np.float64)
        dvals[lids >= 25000] = 1.0
        dmat = dvals.reshape(NQ_U, 128).T
        disc = np.zeros((128, 4 * NQ_U), BF)
        disc[:, :NQ_U] = (L1 * dmat).astype(BF)
        disc[:, NQ_U:2 * NQ_U] = (dmat / L1).astype(BF)
        disc[:, 2 * NQ_U:3 * NQ_U] = dmat.astype(BF)      # L2 folded into M1f
        disc[:, 3 * NQ_U:] = ((L3 / L2) * dmat).astype(BF)

        # P7 offsets
        s0, s1 = int(ownerP_starts[c]), int(ownerP_starts[c + 1])
        idx = ordP[s0:s1]
        ne_c = s1 - s0
        offUP = np.zeros((128, 2 * NCH7), np.int32)
        el = np.arange(ne_c)
        chq = el >> 7
        colU = 16 * (chq >> 3) + (chq & 7)
        offUP[el & 127, colU] = offU_all[idx]
        offUP[el & 127, colU + 8] = offP_all[idx]

        per_core.append(dict(
            featT=featT, embT=embT,
            rows=np.ascontiguousarray(rows[c]),
            cols=np.ascontiguousarray(cols[c]),
            disc=disc, offUP=offUP,
            _idx=idx, _ne=ne_c,
        ))

    shared = dict(
        WufW1c=np.ascontiguousarray((W_uf @ W1c).astype(BF)),
        WpfW1c=np.ascontiguousarray((W_pf @ W1c).astype(BF)),
        W1cb=np.ascontiguousarray(W1c.astype(BF)),
        M1f=np.ascontiguousarray(M1f), M2f=np.ascontiguousarray(M2f),
        bcolU=(b_uf @ W1c).reshape(64, 1).astype(np.float32),
        biasC=((b_pf @ W1c) + b1).reshape(64, 1).astype(np.float32),
        b1mat=np.tile(b1, (128, GQ)).astype(BF),
        cumat=np.tile(constu_f, (128, GQ)).astype(BF),
        cpmat=np.tile(constp_f, (128, 4)).astype(BF),
        iota64=np.tile(np.arange(64, dtype=np.float32), (128, 1)).astype(BF),
        b2col=np.full((128, 1), pb2, np.float32),
    )
    meta = dict(NCHUNKS=NCHUNKS, NCH7=NCH7, quad_subs=quad_subs, PPOS=PPOS,
                zb1=bool(np.all(b1 == 0)), zcu=bool(np.all(constu_f == 0)),
                zcp=bool(np.all(constp_f == 0)))
    return per_core, shared, meta


# --------------------------------------------------------------------------
# device program
# --------------------------------------------------------------------------
def _v3(ap, mid, inner, mid_stride=None, inner_stride=0):
    a = ap.ap
    ms = a[1][0] if mid_stride is None else mid_stride
    return AP(ap.tensor, ap.offset, [list(a[0]), [ms, mid], [inner_stride, inner]])


def build_program(meta):
    NCHUNKS = meta["NCHUNKS"]
    NCH7 = meta["NCH7"]
    quad_subs = meta["quad_subs"]
    PPOS = meta["PPOS"]
    zb1, zcu, zcp = meta["zb1"], meta["zcu"], meta["zcp"]
    assert 0 < PPOS < 64

    nc = bass.Bass("TRN2", target_bir_lowering=False, debug=False,
                   num_devices=N_CORES)
    dp = nc.declare_dram_parameter
    featT_d = dp("featT", [128, SHARD], BF16, isOutput=False)
    embT_d = dp("embT", [64, SHARD], BF16, isOutput=False)
    rows_d = dp("rows", [128, NCHUNKS], I32, isOutput=False)
    cols_d = dp("cols", [128, NCHUNKS], I8, isOutput=False)
    disc_d = dp("disc", [128, 4 * NQ_U], BF16, isOutput=False)
    offUP_d = dp("offUP", [128, 2 * NCH7], I32, isOutput=False)
    WufW1c_d = dp("WufW1c", [128, 64], BF16, isOutput=False)
    WpfW1c_d = dp("WpfW1c", [128, 64], BF16, isOutput=False)
    W1cb_d = dp("W1cb", [64, 64], BF16, isOutput=False)
    M1f_d = dp("M1f", [64, 64], BF16, isOutput=False)
    M2f_d = dp("M2f", [64, 64], BF16, isOutput=False)
    bcolU_d = dp("bcolU", [64, 1], F32, isOutput=False)
    biasC_d = dp("biasC", [64, 1], F32, isOutput=False)
    b1mat_d = dp("b1mat", [128, GQ * 64], BF16, isOutput=False)
    cumat_d = dp("cumat", [128, GQ * 64], BF16, isOutput=False)
    cpmat_d = dp("cpmat", [128, 256], BF16, isOutput=False)
    iota64_d = dp("iota64", [128, 64], BF16, isOutput=False)
    b2col_d = dp("b2col", [128, 1], F32, isOutput=False)
    preds_d = dp("preds", [128, NCH7], F32, isOutput=True)

    # max chunks in one conv batch (for SBUF tile sizing)
    def batch_ranges(qlo, qhi):
        out = []
        q = qlo
        while q < qhi:
            g = min(GQ, qhi - q)
            k0 = quad_subs[q][1][0][0]
            k1 = quad_subs[q + g - 1][1][-1][0] + quad_subs[q + g - 1][1][-1][1]
            out.append((q, g, k0, k1))
            q += g
        return out

    batches_B = batch_ranges(NQ_A, NQ_U)
    batches_A = batch_ranges(0, NQ_A)
    MAXCH = max(k1 - k0 for (_, _, k0, k1) in batches_A + batches_B)

    with tile.TileContext(nc) as tc:
        with tc.tile_pool(name="const", bufs=1) as cp, \
             tc.tile_pool(name="sb", bufs=3) as sb, \
             tc.tile_pool(name="sbm", bufs=2) as sbm, \
             tc.tile_pool(name="ps1", bufs=2, space="PSUM") as ps1, \
             tc.tile_pool(name="ps2", bufs=2, space="PSUM") as ps2, \
             tc.tile_pool(name="pso", bufs=2, space="PSUM") as pso:

            def reg_dge(h):
                mloc = nc.lookup_mloc(h)
                if mloc.table_entry_id is None:
                    mloc.table_entry_id = len(nc.dge_table) + 1
                    nc.dge_table.append(mloc.name)
                return h

            ag1_in = reg_dge(nc.dram_tensor("ag1_in", [UROWS, 64], F8))
            ag2_in = reg_dge(nc.dram_tensor("ag2_in", [UROWS, 64], F8))
            ag3_in = reg_dge(nc.dram_tensor("ag3_in", [PROWS, 64], F8))
            y1_t = reg_dge(nc.dram_tensor("y1_t", [TABU, 64], F8, addr_space="Shared"))
            y2_t = reg_dge(nc.dram_tensor("y2_t", [TABU, 64], F8, addr_space="Shared"))
            ts_t = reg_dge(nc.dram_tensor("ts_t", [UROWS + 8 * PROWS, 64], F8,
                                          addr_space="Shared"))

            # ---- constants ----
            idn = cp.tile([128, 128], F32, tag="idn")
            make_identity(nc, idn[:])
            idn_b = cp.tile([128, 128], BF16, tag="idn_b")
            nc.vector.tensor_copy(out=idn_b[:], in_=idn[:])
            iota64 = cp.tile([128, 64], BF16, tag="iota64")
            nc.sync.dma_start(out=iota64[:], in_=iota64_d[:])
            WufW1c = cp.tile([128, 64], BF16, tag="WufW1c")
            nc.sync.dma_start(out=WufW1c[:], in_=WufW1c_d[:])
            WpfW1c = cp.tile([128, 64], BF16, tag="WpfW1c")
            nc.sync.dma_start(out=WpfW1c[:], in_=WpfW1c_d[:])
            W1cb = cp.tile([64, 64], BF16, tag="W1cb")
            nc.sync.dma_start(out=W1cb[:], in_=W1cb_d[:])
            M1f = cp.tile([64, 64], BF16, tag="M1f")
            nc.sync.dma_start(out=M1f[:], in_=M1f_d[:])
            M2f = cp.tile([64, 64], BF16, tag="M2f")
            nc.sync.dma_start(out=M2f[:], in_=M2f_d[:])
            bcolU = cp.tile([64, 1], F32, tag="bcolU")
            nc.sync.dma_start(out=bcolU[:], in_=bcolU_d[:])
            biasC = cp.tile([64, 1], F32, tag="biasC")
            nc.sync.dma_start(out=biasC[:], in_=biasC_d[:])
            b1mat = cp.tile([128, GQ * 64], BF16, tag="b1mat")
            nc.sync.dma_start(out=b1mat[:], in_=b1mat_d[:])
            cumat = cp.tile([128, GQ * 64], BF16, tag="cumat")
            nc.sync.dma_start(out=cumat[:], in_=cumat_d[:])
            cpmat = cp.tile([128, 256], BF16, tag="cpmat")
            nc.sync.dma_start(out=cpmat[:], in_=cpmat_d[:])
            b2col = cp.tile([128, 1], F32, tag="b2col")
            nc.sync.dma_start(out=b2col[:], in_=b2col_d[:])
            disc = cp.tile([128, 4 * NQ_U], BF16, tag="disc")
            nc.sync.dma_start(out=disc[:], in_=disc_d[:])
            rows_sb = cp.tile([128, NCHUNKS], I32, tag="rows_sb")
            d_rows = nc.sync.dma_start(out=rows_sb[:], in_=rows_d[:])
            cols_i8 = cp.tile([128, NCHUNKS], I8, tag="cols_i8")
            nc.sync.dma_start(out=cols_i8[:], in_=cols_d[:])
            colsb = cp.tile([128, NCHUNKS], BF16, tag="colsb")
            nc.vector.tensor_copy(out=colsb[:], in_=cols_i8[:])
            yown1 = cp.tile([128, NQ_U * 64], F8, tag="yown1")
            yown2 = cp.tile([128, NQ_U * 64], F8, tag="yown2")

            # ================= P1: projection + y1 table + product ts =========
            p1_scatters = [[], []]
            c_scatters = []
            CHQ = 32   # quads per feature-load chunk
            for s0 in range(0, NT, 4):
                nt = min(4, NT - s0)
                is_user = s0 < NQ_U
                if s0 % CHQ == 0:
                    nq = min(CHQ, NT - s0)
                    ftc = sb.tile([128, CHQ * 128], BF16, tag="p1_ftc", bufs=2)
                    nc.sync.dma_start(out=ftc[:, :nq * 128],
                                      in_=featT_d[:, s0 * 128:(s0 + nq) * 128])
                    etc = sb.tile([64, CHQ * 128], BF16, tag="p1_etc", bufs=2)
                    nc.sync.dma_start(out=etc[:, :nq * 128],
                                      in_=embT_d[:, s0 * 128:(s0 + nq) * 128])
                co = (s0 % CHQ) * 128
                w1 = WufW1c if is_user else WpfW1c
                z1p = ps1.tile([64, 512], F32, tag="psA")
                nc.tensor.matmul(out=z1p[:, :nt * 128], lhsT=w1[:],
                                 rhs=ftc[:, co:co + nt * 128], start=True, stop=False)
                nc.tensor.matmul(out=z1p[:, :nt * 128], lhsT=W1cb[:],
                                 rhs=etc[:, co:co + nt * 128], start=False, stop=True)
                if is_user:
                    z1s = sb.tile([64, 512], BF16, tag="p1_z1s")
                    nc.scalar.activation(out=z1s[:, :nt * 128], in_=z1p[:, :nt * 128],
                                         func=AF.Identity, bias=bcolU[:])
                    znm = ps2.tile([128, 256], BF16, tag="znm", bufs=2)
                    for q in range(nt):
                        nc.tensor.transpose(out=znm[:, q * 64:(q + 1) * 64],
                                            in_=z1s[:, q * 128:(q + 1) * 128],
                                            identity=idn_b[:64, :64])
                    nc.vector.tensor_tensor(
                        out=yown1[:, s0 * 64:(s0 + nt) * 64],
                        in0=znm[:, :nt * 64],
                        in1=_v3(disc[:, s0:s0 + nt], nt, 64),
                        op=ALU.mult,
                    )
                    sc = nc.sync.dma_start(
                        out=AP(ag1_in[:].tensor, s0 * 128 * 64,
                               [[64, 128], [8192, nt], [1, 64]]),
                        in_=AP(yown1[:].tensor, yown1[:].offset + s0 * 64,
                               [list(yown1[:].ap[0]), [64, nt], [1, 64]]),
                    )
                    if s0 <= 96:
                        p1_scatters[0].append(sc)
                    if s0 >= 96:
                        p1_scatters[1].append(sc)
                else:
                    rT = sb.tile([64, 512], BF16, tag="p1_rT")
                    nc.scalar.activation(out=rT[:, :nt * 128], in_=z1p[:, :nt * 128],
                                         func=AF.Relu, bias=biasC[:])
                    spp = ps2.tile([128, 256], F32, tag="pp256", bufs=1)
                    for q in range(nt):
                        nc.tensor.matmul(out=spp[:, q * 64:(q + 1) * 64],
                                         lhsT=rT[:, q * 128:(q + 1) * 128],
                                         rhs=M2f[:], start=True, stop=True)
                    sps = sb.tile([128, 256], F8, tag="p1_sps", bufs=6)
                    if zcp:
                        nc.scalar.activation(out=sps[:, :nt * 64],
                                             in_=spp[:, :nt * 64], func=AF.Copy)
                    else:
                        nc.vector.tensor_tensor(out=sps[:, :nt * 64],
                                                in0=spp[:, :nt * 64],
                                                in1=cpmat[:, :nt * 64], op=ALU.add)
                    lp0 = (s0 - NQ_U) * 128
                    c_scatters.append(nc.sync.dma_start(
                        out=AP(ag3_in[:].tensor, lp0 * 64,
                               [[64, 128], [8192, nt], [1, 64]]),
                        in_=AP(sps[:].tensor, sps[:].offset,
                               [list(sps[:].ap[0]), [64, nt], [1, 64]]),
                    ))

            # ================= AllGathers =================
            def ag(src, r0, r1, dst, o0, scatters):
                cc = nc.gpsimd.collective_compute(
                    "AllGather", ALU.bypass,
                    ins=[src[r0:r1, :]],
                    outs=[dst[o0:o0 + N_CORES * (r1 - r0), :]],
                    replica_groups=[list(range(N_CORES))],
                )
                for s in scatters:
                    add_dep_helper(cc.ins, s.ins, sync=True, reason="AG after scatters")
                return cc

            cc1A = ag(ag1_in, 0, 12544, y1_t, 0, p1_scatters[0])

            # ================= conv passes =================
            gstate = {"bi": 0, "gh": {}}

            def conv_batches(layer, y_table, ccdeps, batches, ag_next):
                scatters = []
                gh = gstate["gh"]
                for (q0, g, k0, kend) in batches:
                    bi = gstate["bi"]
                    nck = kend - k0
                    W = quad_subs[q0][0]
                    msg = sbm.tile([128, MAXCH * 64], F8, tag="cv_msg")
                    g_msg = nc.gpsimd.indirect_dma_start(
                        out=msg[:, :nck * 64], out_offset=None,
                        in_=y_table[:],
                        in_offset=IndirectOffsetOnAxis(ap=rows_sb[:, k0:kend], axis=0),
                    )
                    add_dep_helper(g_msg.ins, d_rows.ins, sync=True,
                                   reason="gather reads offsets")
                    for cc in ccdeps:
                        add_dep_helper(g_msg.ins, cc.ins, sync=True,
                                       reason="gather after AG")
                    if bi >= 2 and (bi - 2) in gh:
                        add_dep_helper(g_msg.ins, gh[bi - 2].ins, sync=True,
                                       reason="WAR msg slot")
                    s4 = sbm.tile([128, MAXCH * 64], F8, tag="cv_s4")
                    nc.vector.tensor_tensor(
                        out=AP(s4[:].tensor, s4[:].offset,
                               [list(s4[:].ap[0]), [W, nck], [1, W]]),
                        in0=_v3(colsb[:, k0:kend], nck, W),
                        in1=_v3(iota64[:, :W], nck, W, mid_stride=0,
                                inner_stride=1),
                        op=ALU.is_equal,
                    )
                    opsum = pso.tile([128, GQ * 64], F32, tag="cv_opsum")
                    last_mm = None
                    for qi in range(g):
                        wq, subs = quad_subs[q0 + qi]
                        maxcn = max(cn for (_, cn) in subs)
                        for ci in range(maxcn):
                            for si, (cs, cn) in enumerate(subs):
                                if ci >= cn:
                                    continue
                                k = cs + ci - k0
                                mm = nc.tensor.matmul(
                                    out=opsum[wq * si:wq * si + wq,
                                              qi * 64:(qi + 1) * 64],
                                    lhsT=s4[:, k * W:k * W + wq],
                                    rhs=msg[:, k * 64:(k + 1) * 64],
                                    start=(ci == 0), stop=(ci == cn - 1),
                                    tile_position=(0, wq * si),
                                    skip_group_check=True,
                                )
                                add_dep_helper(mm.ins, g_msg.ins, sync=True,
                                               reason="mm reads msg")
                                last_mm = mm
                    gh[bi] = last_mm
                    gstate["bi"] = bi + 1

                    yown = yown1 if layer == 1 else yown2
                    dblk = NQ_U if layer == 1 else 3 * NQ_U
                    t1 = sb.tile([128, GQ * 64], BF16, tag="cv_t1")
                    nc.vector.tensor_tensor(out=t1[:, :g * 64],
                                            in0=opsum[:, :g * 64],
                                            in1=yown[:, q0 * 64:(q0 + g) * 64],
                                            op=ALU.add)
                    if layer == 2 and zcu:
                        t2 = sb.tile([128, GQ * 64], F8, tag="cv_tsb", bufs=6)
                    else:
                        t2 = sb.tile([128, GQ * 64], BF16, tag="cv_t2")
                    nc.vector.tensor_tensor(
                        out=t2[:, :g * 64], in0=t1[:, :g * 64],
                        in1=_v3(disc[:, dblk + q0:dblk + q0 + g], g, 64),
                        op=ALU.mult)
                    if layer == 1:
                        if zb1:
                            t3 = t2
                        else:
                            t3 = sb.tile([128, GQ * 64], BF16, tag="cv_t3")
                            nc.vector.tensor_tensor(out=t3[:, :g * 64],
                                                    in0=t2[:, :g * 64],
                                                    in1=b1mat[:, :g * 64], op=ALU.add)
                        xr = sb.tile([128, GQ * 64], BF16, tag="cv_xr")
                        nc.scalar.activation(out=xr[:, :g * 64], in_=t3[:, :g * 64],
                                             func=AF.Relu)
                        xs = sb.tile([128, GQ * 64], BF16, tag="cv_xs")
                        nc.vector.tensor_tensor(
                            out=xs[:, :g * 64], in0=xr[:, :g * 64],
                            in1=_v3(disc[:, 2 * NQ_U + q0:2 * NQ_U + q0 + g], g, 64),
                            op=ALU.mult)
                        # per half-batch: transpose + @M1f -> node-major y2 rows
                        h0 = 0
                        while h0 < g:
                            hg = min(4, g - h0)
                            xT = ps1.tile([64, 512], BF16, tag="psAb", bufs=1)
                            for qi in range(hg):
                                nc.tensor.transpose(
                                    out=xT[:, qi * 128:(qi + 1) * 128],
                                    in_=xs[:, (h0 + qi) * 64:(h0 + qi + 1) * 64],
                                    identity=idn_b[:])
                            xTs = sb.tile([64, 512], BF16, tag="cv_xTs")
                            nc.scalar.activation(out=xTs[:, :hg * 128],
                                                 in_=xT[:, :hg * 128], func=AF.Copy)
                            y2p = ps2.tile([128, 256], F32, tag="pp256", bufs=1)
                            for qi in range(hg):
                                nc.tensor.matmul(out=y2p[:, qi * 64:(qi + 1) * 64],
                                                 lhsT=xTs[:, qi * 128:(qi + 1) * 128],
                                                 rhs=M1f[:], start=True, stop=True)
                            nc.scalar.activation(
                                out=yown2[:, (q0 + h0) * 64:(q0 + h0 + hg) * 64],
                                in_=y2p[:, :hg * 64], func=AF.Copy)
                            h0 += hg
                        sc = nc.sync.dma_start(
                            out=AP(ag_next[:].tensor, q0 * 128 * 64,
                                   [[64, 128], [8192, g], [1, 64]]),
                            in_=AP(yown2[:].tensor, yown2[:].offset + q0 * 64,
                                   [list(yown2[:].ap[0]), [64, g], [1, 64]]),
                        )
                    else:
                        if zcu:
                            tsb = t2
                        else:
                            tsb = sb.tile([128, GQ * 64], F8, tag="cv_tsb", bufs=6)
                            nc.vector.tensor_tensor(out=tsb[:, :g * 64],
                                                    in0=t2[:, :g * 64],
                                                    in1=cumat[:, :g * 64], op=ALU.add)
                        sc = nc.sync.dma_start(
                            out=AP(ts_t[:].tensor, q0 * 128 * 64,
                                   [[64, 128], [8192, g], [1, 64]]),
                            in_=AP(tsb[:].tensor, tsb[:].offset,
                                   [list(tsb[:].ap[0]), [64, g], [1, 64]]),
                        )
                    scatters.append(sc)
                return scatters

            cc1B = ag(ag1_in, 12544, UROWS, y1_t, 100352, p1_scatters[1])
            cc3 = ag(ag3_in, 0, PROWS, ts_t, UROWS, c_scatters)
            # conv1 A first -> AG2-A overlaps conv1-B; conv2 B first.
            sc1A = conv_batches(1, y1_t, [cc1A, cc1B], batches_A, ag2_in)
            cc2A = ag(ag2_in, 0, 12544, y2_t, 0, sc1A)
            sc1B = conv_batches(1, y1_t, [cc1A, cc1B], batches_B, ag2_in)
            cc2B = ag(ag2_in, 12544, UROWS, y2_t, 100352, sc1B)
            sc2B = conv_batches(2, y2_t, [cc2A], batches_B, None)
            sc2A = conv_batches(2, y2_t, [cc2A, cc2B], batches_A, None)

            # ================= P7: final pair MLP =================
            offUP_t = cp.tile([128, 2 * NCH7], I32, tag="offUP_t")
            d_off = nc.sync.dma_start(out=offUP_t[:], in_=offUP_d[:])
            pacc = cp.tile([128, NCH7], F32, tag="pacc")
            fence_t = cp.tile([128, 1], F32, tag="fence_t")
            fence = nc.gpsimd.memset(fence_t[:], 0.0)
            for sc in sc2B + sc2A:
                add_dep_helper(fence.ins, sc.ins, sync=True, reason="ts rows done")
            add_dep_helper(fence.ins, cc3.ins, sync=True, reason="AG3 done")

            NIT = NCH7 // (8 * G7)
            p7h = {}
            for it in range(NIT):
                o0 = it * 16 * G7
                h8 = sbm.tile([128, G7 * 512], F8, tag="p7_h8", bufs=3)
                offU_ap = AP(offUP_t[:].tensor, offUP_t[:].offset + o0,
                             [list(offUP_t[:].ap[0]), [16, G7], [1, 8]])
                offP_ap = AP(offUP_t[:].tensor, offUP_t[:].offset + o0 + 8,
                             [list(offUP_t[:].ap[0]), [16, G7], [1, 8]])
                g_t = nc.gpsimd.indirect_dma_start(
                    out=h8[:], out_offset=None, in_=ts_t[:],
                    in_offset=IndirectOffsetOnAxis(ap=offU_ap, axis=0),
                )
                add_dep_helper(g_t.ins, fence.ins, sync=True, reason="after fence")
                add_dep_helper(g_t.ins, d_off.ins, sync=True, reason="offsets loaded")
                g_t2 = nc.gpsimd.indirect_dma_start(
                    out=h8[:], out_offset=None, in_=ts_t[:],
                    in_offset=IndirectOffsetOnAxis(ap=offP_ap, axis=0),
                    compute_op=ALU.add,
                )
                add_dep_helper(g_t2.ins, g_t.ins, sync=True, reason="accum after base")
                if it >= 3 and (it - 3) in p7h:
                    add_dep_helper(g_t.ins, p7h[it - 3].ins, sync=True,
                                   reason="WAR h8")
                hr = sb.tile([128, G7 * 512], BF16, tag="p7_hr")
                rl = nc.scalar.activation(out=hr[:], in_=h8[:], func=AF.Relu)
                add_dep_helper(rl.ins, g_t2.ins, sync=True, reason="reads h8")
                p7h[it] = rl
                redp = sb.tile([128, 8 * G7], BF16, tag="p7_redp")
                redn = sb.tile([128, 8 * G7], BF16, tag="p7_redn")
                with nc.allow_low_precision(reason="logit reduce; /L3 below fp32 noise"):
                    nc.vector.tensor_reduce(
                        out=redp[:],
                        in_=AP(hr[:].tensor, hr[:].offset,
                               [list(hr[:].ap[0]), [64, 8 * G7], [1, PPOS]]),
                        axis=mybir.AxisListType.X, op=ALU.add)
                    nc.vector.tensor_reduce(
                        out=redn[:],
                        in_=AP(hr[:].tensor, hr[:].offset + PPOS,
                               [list(hr[:].ap[0]), [64, 8 * G7], [1, 64 - PPOS]]),
                        axis=mybir.AxisListType.X, op=ALU.add)
                dlt = sb.tile([128, 8 * G7], BF16, tag="p7_dlt")
                nc.vector.tensor_tensor(out=dlt[:], in0=redp[:], in1=redn[:],
                                        op=ALU.subtract)
                sg = sb.tile([128, 8 * G7], F32, tag="p7_sg")
                nc.scalar.activation(out=sg[:], in_=dlt[:], func=AF.Sigmoid,
                                     bias=b2col[:], scale=1.0 / L3)
                nc.vector.tensor_scalar_mul(
                    out=pacc[:, it * 8 * G7:(it + 1) * 8 * G7],
                    in0=sg[:], scalar1=5.0)
            nc.sync.dma_start(out=preds_d[:], in_=pacc[:])

    _split_sync_waits(nc)
    return nc


# --------------------------------------------------------------------------
# runner
# --------------------------------------------------------------------------
def _run(inputs, trace=False):
    per_core, shared, meta = _prepare(inputs)
    nc = build_program(meta)
    in_maps = []
    for c in range(N_CORES):
        m = dict(shared)
        m.update({k: v for k, v in per_core[c].items() if not k.startswith("_")})
        in_maps.append({k: np.ascontiguousarray(v) for k, v in m.items()})
    res = run_bass_kernel_spmd(nc, in_maps, core_ids=list(range(N_CORES)),
                               trace=trace)
    out = np.zeros(NE, np.float32)
    for c in range(N_CORES):
        pc = res.results[c]["preds"]
        idx = per_core[c]["_idx"]
        ne_c = per_core[c]["_ne"]
        el = np.arange(ne_c)
        out[idx] = pc[el & 127, el >> 7]
    return out, res.exec_time_ns


def kernel(**inputs):
    out, _ = _run(inputs, trace=False)
    return out


# revision 23
# speedup vs baseline: 1.0315x; 1.0315x over previous
"""Trainium2 Bass kernel for the bipartite GNN recommender (8 NeuronCores).

v2 — redesigned from the profiled baseline (1.91ms):
- dest sub-tiles of 32 (dense region A) / 64 (region B) nodes stacked
  4/2-per-128-partition PSUM quad: cuts one-hot LDWEIGHTS + DVE build ~4x
- conv2 fused with pred_W1 (host folds conv2_W @ pred_W1 halves; |W2| and
  its sign permutation folded into the ts tables so P7's W2 multiply
  becomes two sign-split reduces)
- own-node y rows stay in SBUF (no self-loop indirect gathers)
- gather offsets loaded once, shared by both conv layers; msg gathers
  batched per 8 dest quads
- AllGathers shrunk: AG1/AG2 user region only, AG3 products only (fired
  right after P1, hidden under conv1/conv2); P7 edges resharded by u%8 so
  the user half of the pair MLP reads a locally-written table
- fp8 tables lambda-scaled out of the subnormal range
"""
import numpy as np
import ml_dtypes

from concourse import bass, mybir, tile
from concourse.bass import AP, IndirectOffsetOnAxis
from concourse.bass_utils import run_bass_kernel_spmd
from concourse.masks import make_identity
from concourse.tile import add_dep_helper

F32 = mybir.dt.float32
BF16 = mybir.dt.bfloat16
F8 = mybir.dt.float8e4
I32 = mybir.dt.int32
I8 = mybir.dt.int8

AF = mybir.ActivationFunctionType
ALU = mybir.AluOpType

N_CORES = 8
NU, NP, NE = 200000, 100000, 1000000
SHARD = 37760          # 25088 user rows + 12672 product rows per core
NQ_U = 196             # user quads (128 rows each) per core
NQ_A = 98              # region A quads (table split A)
NT = 295               # total quads incl products
UROWS = 25088
PROWS = 12672
TABU = 200704          # user table rows (8 * 25088)
L1, L2, L3 = 32.0, 131072.0, 131072.0
GQ = 8                 # dest quads per conv batch
G7 = 4                 # gather-chunk groups per P7 iteration

BF = ml_dtypes.bfloat16


# --------------------------------------------------------------------------
# legalization: this walrus build allows at most 1 sync wait per instruction
# --------------------------------------------------------------------------
def _split_sync_waits(nc, max_waits=1):
    import bass_rust
    for bb in nc.main_func.blocks:
        out = []
        for inst in bb.instructions:
            si = inst.sync_info
            if si is not None and si.on_wait is not None and len(si.on_wait) > max_waits:
                waits = list(si.on_wait)
                keep, extra = waits[-max_waits:], waits[:-max_waits]
                while extra:
                    chunk, extra = extra[:max_waits], extra[max_waits:]
                    nop = bass_rust.InstNoOp(name=f"I-{nc.next_id()}", ins=[], outs=[])
                    nop.engine = inst.engine
                    nop.bass_nofuse = True
                    nop.sync_info = mybir.SyncInfo(on_wait=chunk, on_update=[])
                    nc.register_instruction(nop, overwrite=True)
                    out.append(nop)
                si.on_wait = keep
            out.append(inst)
        del bb.instructions[:]
        for i in out:
            bb.add_instruction(i)


# --------------------------------------------------------------------------
# host-side sharding / layout prep
# --------------------------------------------------------------------------
def _pi_map(j):
    j = np.asarray(j, np.int64)
    c = j % 8
    l = j // 8
    return np.where(l < 12544, c * 12544 + l,
                    100352 + c * 12544 + (l - 12544)).astype(np.int32)


def _prepare(inputs):
    ei = np.asarray(inputs["edge_index"])
    u_idx = ei[0].astype(np.int64)
    p_idx = ei[1].astype(np.int64)

    # ---- conv edges: directed both ways, dest-sharded ----
    src = np.concatenate([u_idx, p_idx])
    dst = np.concatenate([p_idx, u_idx])
    core = (dst % 8).astype(np.int64)
    l = dst // 8
    quad = l >> 7
    sub = quad * 4 + ((l >> 5) & 3)
    NSUB = NQ_U * 4
    colw = (l & 31).astype(np.int8)
    srcpi = _pi_map(src)

    gkey = core * NSUB + sub
    order = np.argsort(gkey, kind="stable")
    gkey_s = gkey[order]
    pi_s = srcpi[order]
    colw_s = colw[order]

    cnt = np.bincount(gkey, minlength=8 * NSUB).reshape(8, NSUB)
    nch = np.maximum((cnt.max(axis=0) + 127) // 128, 1).astype(np.int64)
    chunk_start = np.zeros(NSUB + 1, np.int64)
    chunk_start[1:] = np.cumsum(nch)
    NCHUNKS = int(chunk_start[-1])

    starts = np.searchsorted(gkey_s, np.arange(8 * NSUB))
    pos = np.arange(len(gkey_s)) - starts[gkey_s]
    csub = gkey_s % NSUB
    ccore = gkey_s // NSUB
    ch = chunk_start[csub] + (pos >> 7)
    rows = np.zeros((8, 128, NCHUNKS), np.int32)
    cols = np.full((8, 128, NCHUNKS), -1, np.int8)
    rows[ccore, pos & 127, ch] = pi_s
    cols[ccore, pos & 127, ch] = colw_s

    # quad -> (subtile width, [(chunk_start, nchunks)]) — same for all cores
    quad_subs = []
    for q in range(NQ_U):
        subs = [q * 4 + i for i in range(4)]
        quad_subs.append((32, [(int(chunk_start[s]), int(nch[s])) for s in subs]))

    # ---- degrees / dis ----
    ncnt = np.bincount(dst, minlength=NU + NP).astype(np.float64)
    dis_all = (1.0 / np.sqrt(ncnt + 1.0)).astype(np.float32)

    # ---- P7: edges sharded by u%8 ----
    owner = (u_idx % 8).astype(np.int64)
    ordP = np.argsort(owner, kind="stable")
    cntP = np.bincount(owner, minlength=8)
    NCH7 = int((int(cntP.max()) + 127) // 128)
    NCH7 = ((NCH7 + (8 * G7) - 1) // (8 * G7)) * (8 * G7)
    offU_all = (u_idx // 8).astype(np.int32)
    offP_all = (UROWS + (p_idx % 8) * PROWS + p_idx // 8).astype(np.int32)
    ownerP_starts = np.zeros(9, np.int64)
    ownerP_starts[1:] = np.cumsum(cntP)

    # ---- weights host folds ----
    W_uf = np.asarray(inputs["W_uf"], np.float32)
    W_pf = np.asarray(inputs["W_pf"], np.float32)
    b_uf = np.asarray(inputs["b_uf"], np.float32)
    b_pf = np.asarray(inputs["b_pf"], np.float32)
    W1c = np.asarray(inputs["conv1_W"], np.float32)
    b1 = np.asarray(inputs["conv1_b"], np.float32)
    W2c = np.asarray(inputs["conv2_W"], np.float32)
    b2 = np.asarray(inputs["conv2_b"], np.float32)
    pW1 = np.asarray(inputs["pred_W1"], np.float32)
    pb1 = np.asarray(inputs["pred_b1"], np.float32)
    pW2 = np.asarray(inputs["pred_W2"], np.float32).reshape(64)
    pb2 = float(np.asarray(inputs["pred_b2"]).reshape(()))

    perm = np.argsort(pW2 < 0, kind="stable")
    PPOS = int((pW2 >= 0).sum())
    absw = np.abs(pW2)[perm]
    M1f = (L2 * (W2c @ pW1[:64])[:, perm] * absw[None, :]).astype(BF)
    M2f = (L3 * (W2c @ pW1[64:])[:, perm] * absw[None, :]).astype(BF)
    constu_f = (L3 * (((b2 @ pW1[:64]) + pb1)[perm] * absw)).astype(np.float32)
    constp_f = (L3 * ((b2 @ pW1[64:])[perm] * absw)).astype(np.float32)

    fw = np.ascontiguousarray(np.asarray(inputs["user_features"], np.float32))
    pw = np.ascontiguousarray(np.asarray(inputs["product_features"], np.float32))
    ue = np.asarray(inputs["user_emb"], np.float32)
    pe = np.asarray(inputs["product_emb"], np.float32)

    per_core = []
    for c in range(N_CORES):
        featT = np.zeros((128, SHARD), BF)
        embT = np.zeros((64, SHARD), BF)
        featT[:, :25000] = fw[c::8].T
        featT[:, 25088:37588] = pw[c::8].T
        embT[:, :25000] = ue[c::8].T
        embT[:, 25088:37588] = pe[c::8].T

        # dis per local user row: 4 blocks [L1*dis | dis/L1 | dis | (L3/L2)*dis]
        lids = np.arange(UROWS)
        nid = np.minimum(c + 8 * lids, NU + NP - 1)
        dvals = dis_all[nid].astype(np.float64)
        dvals[lids >= 25000] = 1.0
        dmat = dvals.reshape(NQ_U, 128).T
        disc = np.zeros((128, 4 * NQ_U), BF)
        disc[:, :NQ_U] = (L1 * dmat).astype(BF)
        disc[:, NQ_U:2 * NQ_U] = (dmat / L1).astype(BF)
        disc[:, 2 * NQ_U:3 * NQ_U] = dmat.astype(BF)      # L2 folded into M1f
        disc[:, 3 * NQ_U:] = ((L3 / L2) * dmat).astype(BF)

        # P7 offsets
        s0, s1 = int(ownerP_starts[c]), int(ownerP_starts[c + 1])
        idx = ordP[s0:s1]
        ne_c = s1 - s0
        offUP = np.zeros((128, 2 * NCH7), np.int32)
        el = np.arange(ne_c)
        chq = el >> 7
        colU = 16 * (chq >> 3) + (chq & 7)
        offUP[el & 127, colU] = offU_all[idx]
        offUP[el & 127, colU + 8] = offP_all[idx]

        per_core.append(dict(
            featT=featT, embT=embT,
            rows=np.ascontiguousarray(rows[c]),
            cols=np.ascontiguousarray(cols[c]),
            disc=disc, offUP=offUP,
            _idx=idx, _ne=ne_c,
        ))

    shared = dict(
        WufW1c=np.ascontiguousarray((W_uf @ W1c).astype(BF)),
        WpfW1c=np.ascontiguousarray((W_pf @ W1c).astype(BF)),
        W1cb=np.ascontiguousarray(W1c.astype(BF)),
        M1f=np.ascontiguousarray(M1f), M2f=np.ascontiguousarray(M2f),
        bcolU=(b_uf @ W1c).reshape(64, 1).astype(np.float32),
        biasC=((b_pf @ W1c) + b1).reshape(64, 1).astype(np.float32),
        b1mat=np.tile(b1, (128, GQ)).astype(BF),
        cumat=np.tile(constu_f, (128, GQ)).astype(BF),
        cpmat=np.tile(constp_f, (128, 4)).astype(BF),
        iota64=np.tile(np.arange(64, dtype=np.float32), (128, 1)).astype(BF),
        b2col=np.full((128, 1), pb2, np.float32),
    )
    meta = dict(NCHUNKS=NCHUNKS, NCH7=NCH7, quad_subs=quad_subs, PPOS=PPOS,
                zb1=bool(np.all(b1 == 0)), zcu=bool(np.all(constu_f == 0)),
                zcp=bool(np.all(constp_f == 0)))
    return per_core, shared, meta


# --------------------------------------------------------------------------
# device program
# --------------------------------------------------------------------------
def _v3(ap, mid, inner, mid_stride=None, inner_stride=0):
    a = ap.ap
    ms = a[1][0] if mid_stride is None else mid_stride
    return AP(ap.tensor, ap.offset, [list(a[0]), [ms, mid], [inner_stride, inner]])


def build_program(meta):
    NCHUNKS = meta["NCHUNKS"]
    NCH7 = meta["NCH7"]
    quad_subs = meta["quad_subs"]
    PPOS = meta["PPOS"]
    zb1, zcu, zcp = meta["zb1"], meta["zcu"], meta["zcp"]
    assert 0 < PPOS < 64

    nc = bass.Bass("TRN2", target_bir_lowering=False, debug=False,
                   num_devices=N_CORES)
    dp = nc.declare_dram_parameter
    featT_d = dp("featT", [128, SHARD], BF16, isOutput=False)
    embT_d = dp("embT", [64, SHARD], BF16, isOutput=False)
    rows_d = dp("rows", [128, NCHUNKS], I32, isOutput=False)
    cols_d = dp("cols", [128, NCHUNKS], I8, isOutput=False)
    disc_d = dp("disc", [128, 4 * NQ_U], BF16, isOutput=False)
    offUP_d = dp("offUP", [128, 2 * NCH7], I32, isOutput=False)
    WufW1c_d = dp("WufW1c", [128, 64], BF16, isOutput=False)
    WpfW1c_d = dp("WpfW1c", [128, 64], BF16, isOutput=False)
    W1cb_d = dp("W1cb", [64, 64], BF16, isOutput=False)
    M1f_d = dp("M1f", [64, 64], BF16, isOutput=False)
    M2f_d = dp("M2f", [64, 64], BF16, isOutput=False)
    bcolU_d = dp("bcolU", [64, 1], F32, isOutput=False)
    biasC_d = dp("biasC", [64, 1], F32, isOutput=False)
    b1mat_d = dp("b1mat", [128, GQ * 64], BF16, isOutput=False)
    cumat_d = dp("cumat", [128, GQ * 64], BF16, isOutput=False)
    cpmat_d = dp("cpmat", [128, 256], BF16, isOutput=False)
    iota64_d = dp("iota64", [128, 64], BF16, isOutput=False)
    b2col_d = dp("b2col", [128, 1], F32, isOutput=False)
    preds_d = dp("preds", [128, NCH7], F32, isOutput=True)

    # max chunks in one conv batch (for SBUF tile sizing)
    def batch_ranges(qlo, qhi):
        out = []
        q = qlo
        while q < qhi:
            g = min(GQ, qhi - q)
            k0 = quad_subs[q][1][0][0]
            k1 = quad_subs[q + g - 1][1][-1][0] + quad_subs[q + g - 1][1][-1][1]
            out.append((q, g, k0, k1))
            q += g
        return out

    batches_B = batch_ranges(NQ_A, NQ_U)
    batches_A = batch_ranges(0, NQ_A)
    MAXCH = max(k1 - k0 for (_, _, k0, k1) in batches_A + batches_B)

    with tile.TileContext(nc) as tc:
        with tc.tile_pool(name="const", bufs=1) as cp, \
             tc.tile_pool(name="sb", bufs=3) as sb, \
             tc.tile_pool(name="sbm", bufs=2) as sbm, \
             tc.tile_pool(name="ps1", bufs=2, space="PSUM") as ps1, \
             tc.tile_pool(name="ps2", bufs=2, space="PSUM") as ps2, \
             tc.tile_pool(name="pso", bufs=2, space="PSUM") as pso:

            def reg_dge(h):
                mloc = nc.lookup_mloc(h)
                if mloc.table_entry_id is None:
                    mloc.table_entry_id = len(nc.dge_table) + 1
                    nc.dge_table.append(mloc.name)
                return h

            ag1_in = reg_dge(nc.dram_tensor("ag1_in", [UROWS, 64], F8))
            ag2_in = reg_dge(nc.dram_tensor("ag2_in", [UROWS, 64], F8))
            ag3_in = reg_dge(nc.dram_tensor("ag3_in", [PROWS, 64], F8))
            y1_t = reg_dge(nc.dram_tensor("y1_t", [TABU, 64], F8, addr_space="Shared"))
            y2_t = reg_dge(nc.dram_tensor("y2_t", [TABU, 64], F8, addr_space="Shared"))
            ts_t = reg_dge(nc.dram_tensor("ts_t", [UROWS + 8 * PROWS, 64], F8,
                                          addr_space="Shared"))

            # ---- constants ----
            idn = cp.tile([128, 128], F32, tag="idn")
            make_identity(nc, idn[:])
            idn_b = cp.tile([128, 128], BF16, tag="idn_b")
            nc.vector.tensor_copy(out=idn_b[:], in_=idn[:])
            iota64 = cp.tile([128, 64], BF16, tag="iota64")
            nc.sync.dma_start(out=iota64[:], in_=iota64_d[:])
            WufW1c = cp.tile([128, 64], BF16, tag="WufW1c")
            nc.sync.dma_start(out=WufW1c[:], in_=WufW1c_d[:])
            WpfW1c = cp.tile([128, 64], BF16, tag="WpfW1c")
            nc.sync.dma_start(out=WpfW1c[:], in_=WpfW1c_d[:])
            W1cb = cp.tile([64, 64], BF16, tag="W1cb")
            nc.sync.dma_start(out=W1cb[:], in_=W1cb_d[:])
            M1f = cp.tile([64, 64], BF16, tag="M1f")
            nc.sync.dma_start(out=M1f[:], in_=M1f_d[:])
            M2f = cp.tile([64, 64], BF16, tag="M2f")
            nc.sync.dma_start(out=M2f[:], in_=M2f_d[:])
            bcolU = cp.tile([64, 1], F32, tag="bcolU")
            nc.sync.dma_start(out=bcolU[:], in_=bcolU_d[:])
            biasC = cp.tile([64, 1], F32, tag="biasC")
            nc.sync.dma_start(out=biasC[:], in_=biasC_d[:])
            b1mat = cp.tile([128, GQ * 64], BF16, tag="b1mat")
            nc.sync.dma_start(out=b1mat[:], in_=b1mat_d[:])
            cumat = cp.tile([128, GQ * 64], BF16, tag="cumat")
            nc.sync.dma_start(out=cumat[:], in_=cumat_d[:])
            cpmat = cp.tile([128, 256], BF16, tag="cpmat")
            nc.sync.dma_start(out=cpmat[:], in_=cpmat_d[:])
            b2col = cp.tile([128, 1], F32, tag="b2col")
            nc.sync.dma_start(out=b2col[:], in_=b2col_d[:])
            disc = cp.tile([128, 4 * NQ_U], BF16, tag="disc")
            nc.sync.dma_start(out=disc[:], in_=disc_d[:])
            rows_sb = cp.tile([128, NCHUNKS], I32, tag="rows_sb")
            d_rows = nc.sync.dma_start(out=rows_sb[:], in_=rows_d[:])
            cols_i8 = cp.tile([128, NCHUNKS], I8, tag="cols_i8")
            nc.sync.dma_start(out=cols_i8[:], in_=cols_d[:])
            colsb = cp.tile([128, NCHUNKS], BF16, tag="colsb")
            nc.vector.tensor_copy(out=colsb[:], in_=cols_i8[:])
            yown1 = cp.tile([128, NQ_U * 64], F8, tag="yown1")
            yown2 = cp.tile([128, NQ_U * 64], F8, tag="yown2")

            # ================= P1: projection + y1 table + product ts =========
            p1_scatters = [[], []]
            c_scatters = []
            CHQ = 32   # quads per feature-load chunk
            for s0 in range(0, NT, 4):
                nt = min(4, NT - s0)
                is_user = s0 < NQ_U
                if s0 % CHQ == 0:
                    nq = min(CHQ, NT - s0)
                    ftc = sb.tile([128, CHQ * 128], BF16, tag="p1_ftc", bufs=2)
                    nc.sync.dma_start(out=ftc[:, :nq * 128],
                                      in_=featT_d[:, s0 * 128:(s0 + nq) * 128])
                    etc = sb.tile([64, CHQ * 128], BF16, tag="p1_etc", bufs=2)
                    nc.sync.dma_start(out=etc[:, :nq * 128],
                                      in_=embT_d[:, s0 * 128:(s0 + nq) * 128])
                co = (s0 % CHQ) * 128
                w1 = WufW1c if is_user else WpfW1c
                z1p = ps1.tile([64, 512], F32, tag="psA")
                nc.tensor.matmul(out=z1p[:, :nt * 128], lhsT=w1[:],
                                 rhs=ftc[:, co:co + nt * 128], start=True, stop=False)
                nc.tensor.matmul(out=z1p[:, :nt * 128], lhsT=W1cb[:],
                                 rhs=etc[:, co:co + nt * 128], start=False, stop=True)
                if is_user:
                    z1s = sb.tile([64, 512], BF16, tag="p1_z1s")
                    nc.scalar.activation(out=z1s[:, :nt * 128], in_=z1p[:, :nt * 128],
                                         func=AF.Identity, bias=bcolU[:])
                    znm = ps2.tile([128, 256], BF16, tag="znm", bufs=2)
                    for q in range(nt):
                        nc.tensor.transpose(out=znm[:, q * 64:(q + 1) * 64],
                                            in_=z1s[:, q * 128:(q + 1) * 128],
                                            identity=idn_b[:64, :64])
                    nc.vector.tensor_tensor(
                        out=yown1[:, s0 * 64:(s0 + nt) * 64],
                        in0=znm[:, :nt * 64],
                        in1=_v3(disc[:, s0:s0 + nt], nt, 64),
                        op=ALU.mult,
                    )
                    sc = nc.sync.dma_start(
                        out=AP(ag1_in[:].tensor, s0 * 128 * 64,
                               [[64, 128], [8192, nt], [1, 64]]),
                        in_=AP(yown1[:].tensor, yown1[:].offset + s0 * 64,
                               [list(yown1[:].ap[0]), [64, nt], [1, 64]]),
                    )
                    if s0 <= 96:
                        p1_scatters[0].append(sc)
                    if s0 >= 96:
                        p1_scatters[1].append(sc)
                else:
                    rT = sb.tile([64, 512], BF16, tag="p1_rT")
                    nc.scalar.activation(out=rT[:, :nt * 128], in_=z1p[:, :nt * 128],
                                         func=AF.Relu, bias=biasC[:])
                    spp = ps2.tile([128, 256], F32, tag="pp256", bufs=1)
                    for q in range(nt):
                        nc.tensor.matmul(out=spp[:, q * 64:(q + 1) * 64],
                                         lhsT=rT[:, q * 128:(q + 1) * 128],
                                         rhs=M2f[:], start=True, stop=True)
                    sps = sb.tile([128, 256], F8, tag="p1_sps", bufs=6)
                    if zcp:
                        nc.scalar.activation(out=sps[:, :nt * 64],
                                             in_=spp[:, :nt * 64], func=AF.Copy)
                    else:
                        nc.vector.tensor_tensor(out=sps[:, :nt * 64],
                                                in0=spp[:, :nt * 64],
                                                in1=cpmat[:, :nt * 64], op=ALU.add)
                    lp0 = (s0 - NQ_U) * 128
                    c_scatters.append(nc.sync.dma_start(
                        out=AP(ag3_in[:].tensor, lp0 * 64,
                               [[64, 128], [8192, nt], [1, 64]]),
                        in_=AP(sps[:].tensor, sps[:].offset,
                               [list(sps[:].ap[0]), [64, nt], [1, 64]]),
                    ))

            # ================= AllGathers =================
            def ag(src, r0, r1, dst, o0, scatters):
                cc = nc.gpsimd.collective_compute(
                    "AllGather", ALU.bypass,
                    ins=[src[r0:r1, :]],
                    outs=[dst[o0:o0 + N_CORES * (r1 - r0), :]],
                    replica_groups=[list(range(N_CORES))],
                )
                for s in scatters:
                    add_dep_helper(cc.ins, s.ins, sync=True, reason="AG after scatters")
                return cc

            cc1A = ag(ag1_in, 0, 12544, y1_t, 0, p1_scatters[0])

            # ================= conv passes =================
            gstate = {"bi": 0, "gh": {}}

            def conv_batches(layer, y_table, ccdeps, batches, ag_next):
                scatters = []
                gh = gstate["gh"]
                for (q0, g, k0, kend) in batches:
                    bi = gstate["bi"]
                    nck = kend - k0
                    W = quad_subs[q0][0]
                    msg = sbm.tile([128, MAXCH * 64], F8, tag="cv_msg")
                    g_msg = nc.gpsimd.indirect_dma_start(
                        out=msg[:, :nck * 64], out_offset=None,
                        in_=y_table[:],
                        in_offset=IndirectOffsetOnAxis(ap=rows_sb[:, k0:kend], axis=0),
                    )
                    add_dep_helper(g_msg.ins, d_rows.ins, sync=True,
                                   reason="gather reads offsets")
                    for cc in ccdeps:
                        add_dep_helper(g_msg.ins, cc.ins, sync=True,
                                       reason="gather after AG")
                    if bi >= 2 and (bi - 2) in gh:
                        add_dep_helper(g_msg.ins, gh[bi - 2].ins, sync=True,
                                       reason="WAR msg slot")
                    s4 = sbm.tile([128, MAXCH * 64], F8, tag="cv_s4")
                    nc.vector.tensor_tensor(
                        out=AP(s4[:].tensor, s4[:].offset,
                               [list(s4[:].ap[0]), [W, nck], [1, W]]),
                        in0=_v3(colsb[:, k0:kend], nck, W),
                        in1=_v3(iota64[:, :W], nck, W, mid_stride=0,
                                inner_stride=1),
                        op=ALU.is_equal,
                    )
                    opsum = pso.tile([128, GQ * 64], F32, tag="cv_opsum")
                    last_mm = None
                    for qi in range(g):
                        wq, subs = quad_subs[q0 + qi]
                        maxcn = max(cn for (_, cn) in subs)
                        for ci in range(maxcn):
                            for si, (cs, cn) in enumerate(subs):
                                if ci >= cn:
                                    continue
                                k = cs + ci - k0
                                mm = nc.tensor.matmul(
                                    out=opsum[wq * si:wq * si + wq,
                                              qi * 64:(qi + 1) * 64],
                                    lhsT=s4[:, k * W:k * W + wq],
                                    rhs=msg[:, k * 64:(k + 1) * 64],
                                    start=(ci == 0), stop=(ci == cn - 1),
                                    tile_position=(0, wq * si),
                                    skip_group_check=True,
                                )
                                add_dep_helper(mm.ins, g_msg.ins, sync=True,
                                               reason="mm reads msg")
                                last_mm = mm
                    gh[bi] = last_mm
                    gstate["bi"] = bi + 1

                    yown = yown1 if layer == 1 else yown2
                    dblk = NQ_U if layer == 1 else 3 * NQ_U
                    t1 = sb.tile([128, GQ * 64], BF16, tag="cv_t1")
                    nc.vector.tensor_tensor(out=t1[:, :g * 64],
                                            in0=opsum[:, :g * 64],
                                            in1=yown[:, q0 * 64:(q0 + g) * 64],
                                            op=ALU.add)
                    if layer == 2 and zcu:
                        t2 = sb.tile([128, GQ * 64], F8, tag="cv_tsb", bufs=6)
                    else:
                        t2 = sb.tile([128, GQ * 64], BF16, tag="cv_t2")
                    nc.vector.tensor_tensor(
                        out=t2[:, :g * 64], in0=t1[:, :g * 64],
                        in1=_v3(disc[:, dblk + q0:dblk + q0 + g], g, 64),
                        op=ALU.mult)
                    if layer == 1:
                        if zb1:
                            t3 = t2
                        else:
                            t3 = sb.tile([128, GQ * 64], BF16, tag="cv_t3")
                            nc.vector.tensor_tensor(out=t3[:, :g * 64],
                                                    in0=t2[:, :g * 64],
                                                    in1=b1mat[:, :g * 64], op=ALU.add)
                        xr = sb.tile([128, GQ * 64], BF16, tag="cv_xr")
                        nc.scalar.activation(out=xr[:, :g * 64], in_=t3[:, :g * 64],
                                             func=AF.Relu)
                        xs = sb.tile([128, GQ * 64], BF16, tag="cv_xs")
                        nc.vector.tensor_tensor(
                            out=xs[:, :g * 64], in0=xr[:, :g * 64],
                            in1=_v3(disc[:, 2 * NQ_U + q0:2 * NQ_U + q0 + g], g, 64),
                            op=ALU.mult)
                        # per half-batch: transpose + @M1f -> node-major y2 rows
                        h0 = 0
                        while h0 < g:
                            hg = min(4, g - h0)
                            xT = ps1.tile([64, 512], BF16, tag="psAb", bufs=1)
                            for qi in range(hg):
                                nc.tensor.transpose(
                                    out=xT[:, qi * 128:(qi + 1) * 128],
                                    in_=xs[:, (h0 + qi) * 64:(h0 + qi + 1) * 64],
                                    identity=idn_b[:])
                            xTs = sb.tile([64, 512], BF16, tag="cv_xTs")
                            nc.scalar.activation(out=xTs[:, :hg * 128],
                                                 in_=xT[:, :hg * 128], func=AF.Copy)
                            y2p = ps2.tile([128, 256], F32, tag="pp256", bufs=1)
                            for qi in range(hg):
                                nc.tensor.matmul(out=y2p[:, qi * 64:(qi + 1) * 64],
                                                 lhsT=xTs[:, qi * 128:(qi + 1) * 128],
                                                 rhs=M1f[:], start=True, stop=True)
                            nc.scalar.activation(
                                out=yown2[:, (q0 + h0) * 64:(q0 + h0 + hg) * 64],
                                in_=y2p[:, :hg * 64], func=AF.Copy)
                            h0 += hg
                        sc = nc.sync.dma_start(
                            out=AP(ag_next[:].tensor, q0 * 128 * 64,
                                   [[64, 128], [8192, g], [1, 64]]),
                            in_=AP(yown2[:].tensor, yown2[:].offset + q0 * 64,
                                   [list(yown2[:].ap[0]), [64, g], [1, 64]]),
                        )
                    else:
                        if zcu:
                            tsb = t2
                        else:
                            tsb = sb.tile([128, GQ * 64], F8, tag="cv_tsb", bufs=6)
                            nc.vector.tensor_tensor(out=tsb[:, :g * 64],
                                                    in0=t2[:, :g * 64],
                                                    in1=cumat[:, :g * 64], op=ALU.add)
                        sc = nc.sync.dma_start(
                            out=AP(ts_t[:].tensor, q0 * 128 * 64,
                                   [[64, 128], [8192, g], [1, 64]]),
                            in_=AP(tsb[:].tensor, tsb[:].offset,
                                   [list(tsb[:].ap[0]), [64, g], [1, 64]]),
                        )
                    scatters.append(sc)
                return scatters

            cc1B = ag(ag1_in, 12544, UROWS, y1_t, 100352, p1_scatters[1])
            cc3 = ag(ag3_in, 0, PROWS, ts_t, UROWS, c_scatters)
            # conv1 A first -> AG2-A overlaps conv1-B; conv2 B first.
            sc1A = conv_batches(1, y1_t, [cc1A, cc1B], batches_A, ag2_in)
            cc2A = ag(ag2_in, 0, 12544, y2_t, 0, sc1A)
            sc1B = conv_batches(1, y1_t, [cc1A, cc1B], batches_B, ag2_in)
            cc2B = ag(ag2_in, 12544, UROWS, y2_t, 100352, sc1B)
            sc2B = conv_batches(2, y2_t, [cc2A], batches_B, None)
            sc2A = conv_batches(2, y2_t, [cc2A, cc2B], batches_A, None)

            # ================= P7: final pair MLP =================
            offUP_t = cp.tile([128, 2 * NCH7], I32, tag="offUP_t")
            d_off = nc.sync.dma_start(out=offUP_t[:], in_=offUP_d[:])
            pacc = cp.tile([128, NCH7], F32, tag="pacc")
            fence_t = cp.tile([128, 1], F32, tag="fence_t")
            fence = nc.gpsimd.memset(fence_t[:], 0.0)
            for sc in sc2B + sc2A:
                add_dep_helper(fence.ins, sc.ins, sync=True, reason="ts rows done")
            add_dep_helper(fence.ins, cc3.ins, sync=True, reason="AG3 done")

            NIT = NCH7 // (8 * G7)
            p7h = {}
            for it in range(NIT):
                o0 = it * 16 * G7
                h8 = sbm.tile([128, G7 * 512], F8, tag="p7_h8", bufs=3)
                offU_ap = AP(offUP_t[:].tensor, offUP_t[:].offset + o0,
                             [list(offUP_t[:].ap[0]), [16, G7], [1, 8]])
                offP_ap = AP(offUP_t[:].tensor, offUP_t[:].offset + o0 + 8,
                             [list(offUP_t[:].ap[0]), [16, G7], [1, 8]])
                g_t = nc.gpsimd.indirect_dma_start(
                    out=h8[:], out_offset=None, in_=ts_t[:],
                    in_offset=IndirectOffsetOnAxis(ap=offU_ap, axis=0),
                )
                add_dep_helper(g_t.ins, fence.ins, sync=True, reason="after fence")
                add_dep_helper(g_t.ins, d_off.ins, sync=True, reason="offsets loaded")
                g_t2 = nc.gpsimd.indirect_dma_start(
                    out=h8[:], out_offset=None, in_=ts_t[:],
                    in_offset=IndirectOffsetOnAxis(ap=offP_ap, axis=0),
                    compute_op=ALU.add,
                )
                add_dep_helper(g_t2.ins, g_t.ins, sync=True, reason="accum after base")
                if it >= 3 and (it - 3) in p7h:
                    add_dep_helper(g_t.ins, p7h[it - 3].ins, sync=True,
                                   reason="WAR h8")
                hr = sb.tile([128, G7 * 512], BF16, tag="p7_hr")
                rl = nc.scalar.activation(out=hr[:], in_=h8[:], func=AF.Relu)
                add_dep_helper(rl.ins, g_t2.ins, sync=True, reason="reads h8")
                p7h[it] = rl
                redp = sb.tile([128, 8 * G7], BF16, tag="p7_redp")
                redn = sb.tile([128, 8 * G7], BF16, tag="p7_redn")
                with nc.allow_low_precision(reason="logit reduce; /L3 below fp32 noise"):
                    nc.vector.tensor_reduce(
                        out=redp[:],
                        in_=AP(hr[:].tensor, hr[:].offset,
                               [list(hr[:].ap[0]), [64, 8 * G7], [1, PPOS]]),
                        axis=mybir.AxisListType.X, op=ALU.add)
                    nc.vector.tensor_reduce(
                        out=redn[:],
                        in_=AP(hr[:].tensor, hr[:].offset + PPOS,
                               [list(hr[:].ap[0]), [64, 8 * G7], [1, 64 - PPOS]]),
                        axis=mybir.AxisListType.X, op=ALU.add)
                dlt = sb.tile([128, 8 * G7], BF16, tag="p7_dlt")
                nc.vector.tensor_tensor(out=dlt[:], in0=redp[:], in1=redn[:],
                                        op=ALU.subtract)
                sg = sb.tile([128, 8 * G7], F32, tag="p7_sg")
                nc.scalar.activation(out=sg[:], in_=dlt[:], func=AF.Sigmoid,
                                     bias=b2col[:], scale=1.0 / L3)
                nc.vector.tensor_scalar_mul(
                    out=pacc[:, it * 8 * G7:(it + 1) * 8 * G7],
                    in0=sg[:], scalar1=5.0)
            nc.sync.dma_start(out=preds_d[:], in_=pacc[:])

    _split_sync_waits(nc)
    return nc


# --------------------------------------------------------------------------
# runner
# --------------------------------------------------------------------------
def _run(inputs, trace=False):
    per_core, shared, meta = _prepare(inputs)
    nc = build_program(meta)
    in_maps = []
    for c in range(N_CORES):
        m = dict(shared)
        m.update({k: v for k, v in per_core[c].items() if not k.startswith("_")})
        in_maps.append({k: np.ascontiguousarray(v) for k, v in m.items()})
    res = run_bass_kernel_spmd(nc, in_maps, core_ids=list(range(N_CORES)),
                               trace=trace)
    out = np.zeros(NE, np.float32)
    for c in range(N_CORES):
        pc = res.results[c]["preds"]
        idx = per_core[c]["_idx"]
        ne_c = per_core[c]["_ne"]
        el = np.arange(ne_c)
        out[idx] = pc[el & 127, el >> 7]
    return out, res.exec_time_ns


def kernel(**inputs):
    out, _ = _run(inputs, trace=False)
    return out
